# revision 1
# baseline (speedup 1.0000x reference)
"""Causal single-head attention (B=4, S=2048, D=768) on 8 TRN2 NeuronCores.

Sharding: core (b, h) = batch b, sequence-half h. Each core computes the
attention output for 1024 query rows of one batch. Keys are fed ROTATED by
h*1024 so every core sees the identical score structure: a causal triangle
over the first 1024 key columns plus a dense block over the last 1024 key
columns that is alive only for h=1 (killed via the exp bias input for h=0).

Per-core dataflow (all big matmuls in float32r, 1 cycle/row at N>=256):
  QT[e,i] / KT[e,j] from pre-transposed (host-side) xT and wT inputs,
  V[j,e]; scores per 128-row query tile; exp (+1/sqrt(d) scale and row-sum)
  fused in one ScalarE activation; P tiles transposed on the PE; PV
  accumulated in PSUM; final divide by the row-sum on the way out.
"""

import os
import numpy as np

import concourse.bass as bass
import concourse.mybir as mybir
import concourse.tile as tile
from concourse import bacc
from concourse.bass_utils import run_bass_kernel_spmd

B, S, D = 4, 2048, 768
H = S // 2           # query rows per core
P = 128
ND = D // P          # 6  d/e tiles
NQ = H // P          # 8  query tiles per core
NK = S // P          # 16 key tiles
SCALE = 1.0 / float(np.sqrt(D))
NEG = -10000.0
F32 = mybir.dt.float32
F32R = mybir.dt.float32r

_cached = {}
last_results = None


def _build_nc():
    nc = bacc.Bacc("TRN2", target_bir_lowering=False)

    xT_d = nc.dram_tensor("xT", [D, S], F32, kind="ExternalInput")
    wqT_d = nc.dram_tensor("wqT", [D, D], F32, kind="ExternalInput")
    wkT_d = nc.dram_tensor("wkT", [D, D], F32, kind="ExternalInput")
    wvT_d = nc.dram_tensor("wvT", [D, D], F32, kind="ExternalInput")
    fb_d = nc.dram_tensor("fbias", [P, 1], F32, kind="ExternalInput")
    out_d = nc.dram_tensor("out", [H, D], F32, kind="ExternalOutput")

    with tile.TileContext(nc) as tc:
        with (
            tc.tile_pool(name="qtp", bufs=ND) as qtp,
            tc.tile_pool(name="ktp", bufs=ND) as ktp,
            tc.tile_pool(name="vp", bufs=NK) as vp,
            tc.tile_pool(name="cst", bufs=1) as cst,
        ):
            fb = cst.tile([P, 1], F32)
            nc.sync.dma_start(out=fb[:], in_=fb_d[:, :])

            qts, kts, vs = [], [], []
            # ---- projections (xT/w pools scoped so their SBUF frees) ----
            with (
                tc.tile_pool(name="xp", bufs=ND) as xp,
                tc.tile_pool(name="wp", bufs=7) as wp,
                tc.tile_pool(name="psj", bufs=8, space="PSUM") as psj,
            ):
                def load_w(w_dram):
                    tiles = []
                    for d in range(ND):
                        wt = wp.tile([P, D], F32R, tag="w")
                        nc.sync.dma_start(out=wt[:], in_=w_dram[d * P:(d + 1) * P, :].bitcast(F32R))
                        tiles.append(wt)
                    return tiles

                # interleave w_k with the first x column-chunk (d-paired) so
                # K-proj accumulation trickles with DMA arrival; stream the
                # remaining x chunks column-major
                xs = [xp.tile([P, S], F32R, name=f"xt{d}", tag="xt") for d in range(ND)]
                wk = []
                for d in range(ND):
                    wt = wp.tile([P, D], F32R, tag="w")
                    nc.sync.dma_start(out=wt[:], in_=wkT_d[d * P:(d + 1) * P, :].bitcast(F32R))
                    wk.append(wt)
                    nc.sync.dma_start(
                        out=xs[d][:, 0:512],
                        in_=xT_d[d * P:(d + 1) * P, 0:512].bitcast(F32R))
                for c0 in range(512, S, 512):
                    for d in range(ND):
                        nc.sync.dma_start(
                            out=xs[d][:, c0:c0 + 512],
                            in_=xT_d[d * P:(d + 1) * P, c0:c0 + 512].bitcast(F32R))

                # KT[e,j] = sum_d wkT[d,e]^T xT[d,j]
                for et in range(ND):
                    kt = ktp.tile([P, S], F32R)
                    kts.append(kt)
                    for c0 in range(0, S, 512):
                        acc = psj.tile([P, 512], F32, tag="ps")
                        for d in range(ND):
                            nc.tensor.matmul(
                                acc[:],
                                wk[d][:, et * P:(et + 1) * P],
                                xs[d][:, c0:c0 + 512],
                                start=(d == 0), stop=(d == ND - 1),
                            )
                        nc.vector.tensor_copy(kt[:, c0:c0 + 512], acc[:])

                # V[j,e] = sum_d xT[d,j]^T wvT[d,e]
                wv = load_w(wvT_d)
                for jt in range(NK):
                    v = vp.tile([P, D + 2], F32R)
                    vs.append(v)
                    for e0, ew in ((0, 512), (512, 256)):
                        acc = psj.tile([P, 512], F32, tag="ps")
                        for d in range(ND):
                            nc.tensor.matmul(
                                acc[:, :ew],
                                xs[d][:, jt * P:(jt + 1) * P],
                                wv[d][:, e0:e0 + ew],
                                start=(d == 0), stop=(d == ND - 1),
                            )
                        nc.vector.tensor_copy(v[:, e0:e0 + ew], acc[:, :ew])
                    ones = nc.const_aps.tensor(1.0, (P, 2), F32)
                    nc.vector.tensor_copy(v[:, D:D + 2], ones)

                # QT[e,i] = sum_d wqT[d,e]^T xT[d,i]  for i in [0, H)
                wq = load_w(wqT_d)
                for et in range(ND):
                    qt = qtp.tile([P, H], F32R)
                    qts.append(qt)
                    for qc0, xc0 in ((0, 0), (512, 1024)):
                        acc = psj.tile([P, 512], F32, tag="ps")
                        for d in range(ND):
                            nc.tensor.matmul(
                                acc[:],
                                wq[d][:, et * P:(et + 1) * P],
                                xs[d][:, xc0:xc0 + 512],
                                start=(d == 0), stop=(d == ND - 1),
                            )
                        nc.vector.tensor_copy(qt[:, qc0:qc0 + 512], acc[:])

            # ---- attention: transposed scores over 512-query quads ----
            # scoresT[j, i] tiles [128, 512] (four query tiles per pass; N=512
            # amortizes the per-matmul LDWEIGHTS, which is the PE limiter at
            # N=256). Phase A: scores + exp into PT tiles for every live key
            # tile of the quad. Phase B: PV per 128-query half. The ones
            # columns of V give the softmax denominator in pv[:, D].
            with (
                tc.tile_pool(name="ptp", bufs=24) as ptp,
                tc.tile_pool(name="dgp", bufs=3) as dgp,
                tc.tile_pool(name="sgp", bufs=4) as sgp,
                tc.tile_pool(name="op", bufs=3) as op,
                tc.tile_pool(name="ps", bufs=4, space="PSUM") as ps_pool,
                tc.tile_pool(name="pspv", bufs=2, space="PSUM") as pspv_pool,
            ):
                for q in range(2):
                    qc = 512 * q              # QT column base
                    i_rot0 = 0 if q == 0 else 1024
                    if q == 0:
                        jts = list(range(4)) + list(range(12, NK))
                        tri = set(range(4))
                    else:
                        jts = list(range(12)) + list(range(12, NK))
                        tri = set(range(8, 12))
                    pts = {}
                    for jt in jts:
                        st = ps_pool.tile([P, 512], F32, tag="st")
                        for et in range(ND):
                            nc.tensor.matmul(
                                st[:],
                                kts[et][:, jt * P:(jt + 1) * P],
                                qts[et][:, qc:qc + 512],
                                start=(et == 0), stop=(et == ND - 1),
                            )
                        pt = ptp.tile([P, 512], F32R, tag="pt")
                        pts[jt] = pt
                        if jt in tri:
                            dg = dgp.tile([P, 512], F32, tag="dg")
                            nc.vector.tensor_copy(dg[:], st[:])
                            # keep where (i_rot0 + f) - (128*jt + p) >= 0
                            nc.gpsimd.affine_select(
                                out=dg[:], in_=dg[:],
                                compare_op=mybir.AluOpType.is_ge,
                                fill=NEG, base=i_rot0 - P * jt,
                                pattern=[[1, 512]], channel_multiplier=-1,
                            )
                            nc.scalar.activation(
                                pt[:], dg[:], mybir.ActivationFunctionType.Exp,
                                bias=0.0, scale=SCALE,
                            )
                        else:
                            nc.scalar.activation(
                                pt[:], st[:], mybir.ActivationFunctionType.Exp,
                                bias=(fb[:, 0:1] if jt >= 12 else 0.0), scale=SCALE,
                            )
                    for half in range(4):
                        pv = pspv_pool.tile([P, D + 2], F32, tag="pv")
                        h0 = half * P
                        for idx, jt in enumerate(jts):
                            for e0, ew in ((0, 512), (512, D + 2 - 512)):
                                nc.tensor.matmul(
                                    pv[:, e0:e0 + ew],
                                    pts[jt][:, h0:h0 + P],
                                    vs[jt][:, e0:e0 + ew],
                                    start=(idx == 0), stop=(idx == len(jts) - 1),
                                )
                        rcp = sgp.tile([P, 1], F32, tag="rcp")
                        nc.vector.reciprocal(rcp[:], pv[:, D:D + 1])
                        o = op.tile([P, D], F32, tag="o")
                        nc.vector.tensor_scalar_mul(o[:], pv[:, :D], rcp[:])
                        r0 = qc + h0
                        nc.sync.dma_start(out=out_d[r0:r0 + P, :], in_=o[:])

    nc.compile()
    return nc


def _get_nc():
    if "nc" not in _cached:
        _cached["nc"] = _build_nc()
    return _cached["nc"]


def kernel(x, w_q, w_k, w_v):
    global last_results
    x = np.ascontiguousarray(np.asarray(x, dtype=np.float32))
    wqT = np.ascontiguousarray(np.asarray(w_q, dtype=np.float32).T)
    wkT = np.ascontiguousarray(np.asarray(w_k, dtype=np.float32).T)
    wvT = np.ascontiguousarray(np.asarray(w_v, dtype=np.float32).T)

    nc = _get_nc()
    in_maps = []
    for core in range(8):
        b, h = core // 2, core % 2
        r = 512 * h
        rot = np.concatenate([x[b, r:], x[b, :r]], axis=0)
        in_maps.append({
            "xT": np.ascontiguousarray(rot.T),
            "wqT": wqT, "wkT": wkT, "wvT": wvT,
            "fbias": np.full((P, 1), 0.0 if h == 1 else NEG, np.float32),
        })

    trace = bool(int(os.environ.get("KERNEL_TRACE", "0")))
    res = run_bass_kernel_spmd(nc, in_maps, core_ids=list(range(8)), trace=trace)
    last_results = res

    out = np.empty((B, S, D), np.float32)
    for core in range(8):
        b, h = core // 2, core % 2
        r = 512 * h
        o = res.results[core]["out"]
        out[b, r:r + 512] = o[0:512]
        out[b, 1024 + r:1024 + r + 512] = o[512:1024]
    return out



# revision 2
# speedup vs baseline: 1.1417x; 1.1417x over previous
"""Causal single-head attention (B=4, S=2048, D=768) on 8 TRN2 NeuronCores.

Sharding: core (b, h) = batch b, sequence-half h. Each core computes the
attention output for 1024 query rows of one batch. Keys are fed ROTATED by
h*512 so every core sees the identical score structure: a causal triangle
plus dense blocks, with the h-dead key tiles killed via the exp bias input.

v2: all operands bf16 (host-cast) — halves DMA traffic and SBUF footprint;
PV accumulates only over the per-half live key-tile sets {12..15, 0..i(+8)}
(identical across cores in rotated space), dropping 12 of 96 PV pairs.
"""

import os
import numpy as np
import ml_dtypes

import concourse.bass as bass
import concourse.mybir as mybir
import concourse.tile as tile
from concourse import bacc
from concourse.bass_utils import run_bass_kernel_spmd

B, S, D = 4, 2048, 768
H = S // 2           # query rows per core
P = 128
ND = D // P          # 6  d/e tiles
NQ = H // P          # 8  query tiles per core
NK = S // P          # 16 key tiles
SCALE = 1.0 / float(np.sqrt(D))
NEG = -10000.0
F32 = mybir.dt.float32
BF16 = mybir.dt.bfloat16
NPBF16 = np.dtype(ml_dtypes.bfloat16)

_cached = {}
last_results = None


def _build_nc():
    nc = bacc.Bacc("TRN2", target_bir_lowering=False)

    xT_d = nc.dram_tensor("xT", [D, S], BF16, kind="ExternalInput")
    wqT_d = nc.dram_tensor("wqT", [D, D], BF16, kind="ExternalInput")
    wkT_d = nc.dram_tensor("wkT", [D, D], BF16, kind="ExternalInput")
    wvT_d = nc.dram_tensor("wvT", [D, D], BF16, kind="ExternalInput")
    fb_d = nc.dram_tensor("fbias", [P, 1], F32, kind="ExternalInput")
    out_d = nc.dram_tensor("out", [H, D], BF16, kind="ExternalOutput")

    with tile.TileContext(nc) as tc:
        with (
            tc.tile_pool(name="qtp", bufs=ND) as qtp,
            tc.tile_pool(name="ktp", bufs=ND) as ktp,
            tc.tile_pool(name="vp", bufs=NK) as vp,
            tc.tile_pool(name="cst", bufs=1) as cst,
        ):
            fb = cst.tile([P, 1], F32)
            nc.sync.dma_start(out=fb[:], in_=fb_d[:, :])

            qts, kts, vs = [], [], []
            # ---- projections (xT/w pools scoped so their SBUF frees) ----
            with (
                tc.tile_pool(name="xp", bufs=ND) as xp,
                tc.tile_pool(name="wp", bufs=7) as wp,
                tc.tile_pool(name="psj", bufs=8, space="PSUM") as psj,
            ):
                def load_w(w_dram):
                    tiles = []
                    for d in range(ND):
                        wt = wp.tile([P, D], BF16, tag="w")
                        nc.sync.dma_start(out=wt[:], in_=w_dram[d * P:(d + 1) * P, :])
                        tiles.append(wt)
                    return tiles

                # interleave w_k with the first x column-chunk (d-paired) so
                # K-proj accumulation trickles with DMA arrival; stream the
                # remaining x chunks column-major
                xs = [xp.tile([P, S], BF16, name=f"xt{d}", tag="xt") for d in range(ND)]
                wk = []
                for d in range(ND):
                    wt = wp.tile([P, D], BF16, tag="w")
                    nc.sync.dma_start(out=wt[:], in_=wkT_d[d * P:(d + 1) * P, :])
                    wk.append(wt)
                    nc.sync.dma_start(
                        out=xs[d][:, 0:512],
                        in_=xT_d[d * P:(d + 1) * P, 0:512])
                for c0 in range(512, S, 512):
                    for d in range(ND):
                        nc.sync.dma_start(
                            out=xs[d][:, c0:c0 + 512],
                            in_=xT_d[d * P:(d + 1) * P, c0:c0 + 512])

                # KT[e,j] = sum_d wkT[d,e]^T xT[d,j]
                for et in range(ND):
                    kt = ktp.tile([P, S], BF16)
                    kts.append(kt)
                    for c0 in range(0, S, 512):
                        acc = psj.tile([P, 512], F32, tag="ps")
                        for d in range(ND):
                            nc.tensor.matmul(
                                acc[:],
                                wk[d][:, et * P:(et + 1) * P],
                                xs[d][:, c0:c0 + 512],
                                start=(d == 0), stop=(d == ND - 1),
                            )
                        nc.vector.tensor_copy(kt[:, c0:c0 + 512], acc[:])

                # V[j,e] = sum_d xT[d,j]^T wvT[d,e]
                wv = load_w(wvT_d)
                for jt in range(NK):
                    v = vp.tile([P, D + 2], BF16)
                    vs.append(v)
                    for e0, ew in ((0, 512), (512, 256)):
                        acc = psj.tile([P, 512], F32, tag="ps")
                        for d in range(ND):
                            nc.tensor.matmul(
                                acc[:, :ew],
                                xs[d][:, jt * P:(jt + 1) * P],
                                wv[d][:, e0:e0 + ew],
                                start=(d == 0), stop=(d == ND - 1),
                            )
                        nc.vector.tensor_copy(v[:, e0:e0 + ew], acc[:, :ew])
                    ones = nc.const_aps.tensor(1.0, (P, 2), BF16)
                    nc.vector.tensor_copy(v[:, D:D + 2], ones)

                # QT[e,i] = sum_d wqT[d,e]^T xT[d,i]  for i in [0, H)
                wq = load_w(wqT_d)
                for et in range(ND):
                    qt = qtp.tile([P, H], BF16)
                    qts.append(qt)
                    for qc0, xc0 in ((0, 0), (512, 1024)):
                        acc = psj.tile([P, 512], F32, tag="ps")
                        for d in range(ND):
                            nc.tensor.matmul(
                                acc[:],
                                wq[d][:, et * P:(et + 1) * P],
                                xs[d][:, xc0:xc0 + 512],
                                start=(d == 0), stop=(d == ND - 1),
                            )
                        nc.vector.tensor_copy(qt[:, qc0:qc0 + 512], acc[:])

            # ---- attention: transposed scores over 512-query quads ----
            # scoresT[j, i] tiles [128, 512]. Phase A: scores + exp into PT
            # tiles for every live key tile of the quad. Phase B: PV per
            # 128-query half over only that half's live key tiles (rotated
            # sets {12..15} + {0..i} are core-independent). The ones columns
            # of V give the softmax denominator in pv[:, D].
            with (
                tc.tile_pool(name="ptp", bufs=24) as ptp,
                tc.tile_pool(name="dgp", bufs=3) as dgp,
                tc.tile_pool(name="sgp", bufs=4) as sgp,
                tc.tile_pool(name="op", bufs=3) as op,
                tc.tile_pool(name="ps", bufs=4, space="PSUM") as ps_pool,
                tc.tile_pool(name="pspv", bufs=2, space="PSUM") as pspv_pool,
            ):
                for q in range(2):
                    qc = 512 * q              # QT column base
                    i_rot0 = 0 if q == 0 else 1024
                    if q == 0:
                        jts = list(range(4)) + list(range(12, NK))
                        tri = set(range(4))
                    else:
                        jts = list(range(12)) + list(range(12, NK))
                        tri = set(range(8, 12))
                    pts = {}
                    for jt in jts:
                        st = ps_pool.tile([P, 512], F32, tag="st")
                        for et in range(ND):
                            nc.tensor.matmul(
                                st[:],
                                kts[et][:, jt * P:(jt + 1) * P],
                                qts[et][:, qc:qc + 512],
                                start=(et == 0), stop=(et == ND - 1),
                            )
                        pt = ptp.tile([P, 512], BF16, tag="pt")
                        pts[jt] = pt
                        if jt in tri:
                            dg = dgp.tile([P, 512], F32, tag="dg")
                            nc.vector.tensor_copy(dg[:], st[:])
                            # keep where (i_rot0 + f) - (128*jt + p) >= 0
                            nc.gpsimd.affine_select(
                                out=dg[:], in_=dg[:],
                                compare_op=mybir.AluOpType.is_ge,
                                fill=NEG, base=i_rot0 - P * jt,
                                pattern=[[1, 512]], channel_multiplier=-1,
                            )
                            nc.scalar.activation(
                                pt[:], dg[:], mybir.ActivationFunctionType.Exp,
                                bias=0.0, scale=SCALE,
                            )
                        else:
                            nc.scalar.activation(
                                pt[:], st[:], mybir.ActivationFunctionType.Exp,
                                bias=(fb[:, 0:1] if jt >= 12 else 0.0), scale=SCALE,
                            )
                    for half in range(4):
                        # live key tiles for this 128-query half, in rotated
                        # key space (identical for h=0/h=1 cores)
                        jts_half = list(range(12, NK)) + list(range(half + 1 + 8 * q))
                        pv = pspv_pool.tile([P, D + 2], F32, tag="pv")
                        h0 = half * P
                        for idx, jt in enumerate(jts_half):
                            for e0, ew in ((0, 512), (512, D + 2 - 512)):
                                nc.tensor.matmul(
                                    pv[:, e0:e0 + ew],
                                    pts[jt][:, h0:h0 + P],
                                    vs[jt][:, e0:e0 + ew],
                                    start=(idx == 0), stop=(idx == len(jts_half) - 1),
                                )
                        rcp = sgp.tile([P, 1], F32, tag="rcp")
                        nc.vector.reciprocal(rcp[:], pv[:, D:D + 1])
                        o = op.tile([P, D], BF16, tag="o")
                        nc.vector.tensor_scalar_mul(o[:], pv[:, :D], rcp[:])
                        r0 = qc + h0
                        nc.sync.dma_start(out=out_d[r0:r0 + P, :], in_=o[:])

    nc.compile()
    return nc


def _get_nc():
    if "nc" not in _cached:
        _cached["nc"] = _build_nc()
    return _cached["nc"]


def kernel(x, w_q, w_k, w_v):
    global last_results
    x = np.asarray(x, dtype=np.float32)
    wqT = np.ascontiguousarray(np.asarray(w_q, dtype=np.float32).T.astype(NPBF16))
    wkT = np.ascontiguousarray(np.asarray(w_k, dtype=np.float32).T.astype(NPBF16))
    wvT = np.ascontiguousarray(np.asarray(w_v, dtype=np.float32).T.astype(NPBF16))

    nc = _get_nc()
    in_maps = []
    for core in range(8):
        b, h = core // 2, core % 2
        r = 512 * h
        rot = np.concatenate([x[b, r:], x[b, :r]], axis=0)
        in_maps.append({
            "xT": np.ascontiguousarray(rot.T.astype(NPBF16)),
            "wqT": wqT, "wkT": wkT, "wvT": wvT,
            "fbias": np.full((P, 1), 0.0 if h == 1 else NEG, np.float32),
        })

    trace = bool(int(os.environ.get("KERNEL_TRACE", "0")))
    res = run_bass_kernel_spmd(nc, in_maps, core_ids=list(range(8)), trace=trace)
    last_results = res

    out = np.empty((B, S, D), np.float32)
    for core in range(8):
        b, h = core // 2, core % 2
        r = 512 * h
        o = np.asarray(res.results[core]["out"]).astype(np.float32)
        out[b, r:r + 512] = o[0:512]
        out[b, 1024 + r:1024 + r + 512] = o[512:1024]
    return out
